# revision 18
# baseline (speedup 1.0000x reference)
"""Trainium2 Bass kernel for nn_AdvancedWaveletDecomp (self-contained).

Pure data parallel over batch B=32 across 8 NeuronCores (4 samples/core).
On each core, 2 samples are packed per 128-partition group (rows 0:64 =
sample A channels, 64:128 = sample B), giving 2 groups per core.

 - ODE (10 RK4 steps): K=3 channel-mixing convs = 3 accumulating 128x128
   block-diagonal matmuls over shifted views of halo-padded SBUF tiles.
   RK4 is refolded so stage inputs are t_i = a_i*(z + c_i*u_j) with
   host-precomputed constants (a_i folded into per-stage conv1 weights).
   State z stays fp32; stage rhs / gelu / tanh tensors are bf16.
 - Dynamic wavelet filters: per-level stat MLP in fp32 matmuls (columns =
   samples); the [16,4] filter block is PE-transposed, broadcast into
   per-sample [128,8] tables, expanded to diagonal lhsT matrices, and the
   8-tap filters run as accumulating f32r diagonal matmuls over shifted
   edge-padded windows.
 - Cross-scale fusion: 1x1 att convs + K=3 zero-padded gate convs as f32r
   matmuls, sigmoid/gelu on ACT, gating on DVE.
 - ortho and the tiny reshapes/scalar reductions run on host numpy.
"""
from contextlib import ExitStack

import numpy as np
import ml_dtypes

import concourse.bass as bass
import concourse.bacc as bacc
import concourse.tile as tile
import concourse.mybir as mybir
import bass_rust
from concourse.bass_utils import run_bass_kernel_spmd

B, C, L = 32, 64, 4096
LEVEL, FL, DIM, HID, MID = 3, 8, 64, 64, 16
REG = 0.01
ODE_H, ODE_STEPS = 0.1, 10
NCORES = 8
BS = B // NCORES     # samples per core
G = BS // 2          # 2-sample groups per core

F32 = mybir.dt.float32
F32R = mybir.dt.float32r
BF16 = mybir.dt.bfloat16
F16 = mybir.dt.float16

IL, IR = 4, 4 + L    # interior columns of padded tiles
W = L + 8            # padded tile width

AF = mybir.ActivationFunctionType
OP = mybir.AluOpType

TileCtx = tile.TileContext

# power-of-2 scaling keeps fp16 wavelet operands out of subnormals
WS = [1.0, 64.0, 2097152.0]        # approx storage scale per level
WD = [64.0, 524288.0, 70368744177664.0]  # det psum scale per level (2^6,2^19,2^46)



def _rk4_constants(damp: float):
    """Exact z' coefficients over [z, u1..u4] plus truncated stage scales."""
    h = ODE_H
    z = np.array([1.0, 0, 0, 0, 0], np.float64)
    e = np.eye(5)
    t1 = z
    k1 = e[1] - damp * t1
    t2 = z + (h / 2) * k1
    k2 = e[2] - damp * t2
    t3 = z + (h / 2) * k2
    k3 = e[3] - damp * t3
    t4 = z + h * k3
    k4 = e[4] - damp * t4
    zc = z + (h / 6) * (k1 + 2 * k2 + 2 * k3 + k4)
    A, B1, B2, B3, B4 = [float(v) for v in zc]
    a2 = 1.0 - damp * h / 2
    a3 = 1.0 - damp * h / 2 * a2
    a4 = 1.0 - damp * h * a3
    return dict(A=A, Bs=[B1, B2, B3, B4], stage_scale=[1.0, a2, a3, a4],
                stage_c=[None, (h / 2) / a2, (h / 2) / a3, h / a4])


def build_program(damp: float, steps: int = ODE_STEPS):
    ks = _rk4_constants(damp)
    A = ks["A"]
    B1, B2, B3, B4 = ks["Bs"]
    nc = bacc.Bacc("TRN2", target_bir_lowering=False, debug=False)

    # ---------------- DRAM I/O ----------------
    def din(name, shape, dt):
        return nc.dram_tensor(name, list(shape), dt, kind="ExternalInput").ap()

    d_x = din("x", [G, 128, L], F32)
    d_c1w = din("c1w", [128, 4 * 3 * 128], F16)       # [cin_s, (stage,k,cout_s)]
    d_c2w = din("c2w", [128, 3 * 128], F16)           # [din_s, (k,cout_s)]
    d_c1b = din("c1b", [128, 1], F32)
    d_c2b = din("c2b", [128, 1], F32)
    d_statw = din("statw", [128, 128], F32)
    d_statb = din("statb", [128, 1], F32)
    d_gen1w = din("gen1w", [128, 128], F32)            # both halves = gen1_w.T
    d_gen1b = din("gen1b", [128, 1], F32)
    d_gen2w = din("gen2w", [128, 16], F32)
    d_gen2b = din("gen2b", [16, 1], F32)
    d_gatew = din("gatew", [128, 3 * 3 * 128], F16)    # [cin_s, (lev,k,cout_s)]
    d_gateb = din("gateb", [128, 3], F32)
    d_attw1 = din("attw1", [128, 2 * 32], F16)         # [cin_s, (lev, m_s)]
    d_attb1 = din("attb1", [32, 2], F32)
    d_attw2 = din("attw2", [32, 2 * 128], F16)        # [m_s, (lev, cout_s)]
    d_attb2 = din("attb2", [128, 2], F32)
    d_id = din("idmat", [128, 128], F32)

    d_cur = nc.dram_tensor("out_cur", [G, 128, L], F32, kind="ExternalOutput").ap()
    d_enh = [nc.dram_tensor(f"out_enh{i}", [G, 128, L], F32,
                            kind="ExternalOutput").ap() for i in range(3)]
    d_g = nc.dram_tensor("out_g", [LEVEL, 16, 2 * G], F32,
                         kind="ExternalOutput").ap()
    d_det = nc.dram_tensor("det_scr", [2, G, 128, L], F32).ap()  # lev 0,1 scratch

    def bcast_cols(dst, src_col, n):
        # replicate one column into n columns (edge padding)
        nc.vector.tensor_copy(dst, src_col.broadcast_to((src_col.shape[0], n)))

    with TileCtx(nc) as tc, ExitStack() as top:
        const = top.enter_context(tc.tile_pool(name="const", bufs=1))
        state = top.enter_context(tc.tile_pool(name="state", bufs=1))

        # ---- load constants ----
        def cload(dname, dram_ap, shape, dt):
            t = const.tile(shape, dt, name=f"c_{dname}", tag=f"c_{dname}")
            nc.sync.dma_start(t[:], dram_ap)
            return t

        c1w = cload("c1w", d_c1w, [128, 4 * 3 * 128], F16)
        c2w = cload("c2w", d_c2w, [128, 3 * 128], F16)
        c1b = cload("c1b", d_c1b, [128, 1], F32)
        c2b = cload("c2b", d_c2b, [128, 1], F32)
        statw = cload("statw", d_statw, [128, 128], F32)
        statb = cload("statb", d_statb, [128, 1], F32)
        gen1w = cload("gen1w", d_gen1w, [128, 128], F32)
        gen1b = cload("gen1b", d_gen1b, [128, 1], F32)
        gen2w = cload("gen2w", d_gen2w, [128, 16], F32)
        gen2b = cload("gen2b", d_gen2b, [16, 1], F32)
        gatew = cload("gatew", d_gatew, [128, 3 * 3 * 128], F16)
        gateb = cload("gateb", d_gateb, [128, 3], F32)
        attw1 = cload("attw1", d_attw1, [128, 2 * 32], F16)
        attb1 = cload("attb1", d_attb1, [32, 2], F32)
        attw2 = cload("attw2", d_attw2, [32, 2 * 128], F16)
        attb2 = cload("attb2", d_attb2, [128, 2], F32)
        idm = cload("idmat", d_id, [128, 128], F32)

        def c1w_sl(stage, k):
            o = (stage * 3 + k) * 128
            return c1w[:, o:o + 128]

        def c2w_sl(k):
            return c2w[:, k * 128:(k + 1) * 128]

        def gatew_sl(lev, k):
            o = (lev * 3 + k) * 128
            return gatew[:, o:o + 128]

        # ---- persistent state tiles ----
        z = [state.tile([128, W], F32, name=f"z{g}", tag=f"z{g}")
             for g in range(G)]
        for g in range(G):
            nc.sync.dma_start(z[g][:, IL:IR], d_x[g])
            nc.vector.tensor_copy(z[g][:, IL - 1:IL], z[g][:, IL:IL + 1])
            nc.vector.tensor_copy(z[g][:, IR:IR + 1], z[g][:, IR - 1:IR])

        # ================= ODE =================
        with ExitStack() as ode_ctx:
            ps_pool = ode_ctx.enter_context(
                tc.tile_pool(name="odeps", bufs=4, space="PSUM"))
            zp16 = ode_ctx.enter_context(tc.tile_pool(name="z16p", bufs=1))
            hp = ode_ctx.enter_context(tc.tile_pool(name="hp", bufs=2))
            tpp = ode_ctx.enter_context(tc.tile_pool(name="tpp", bufs=2))
            up = ode_ctx.enter_context(tc.tile_pool(name="up", bufs=10))

            z16 = [zp16.tile([128, W], F16, name=f"z16_{g}", tag=f"z16_{g}")
                   for g in range(G)]

            def conv_block(w_sl_fn, rhs, dst_tile, func, bias_ap):
                # full-width K=3 conv + activation; rhs/dst are [128, W] tiles
                for q in range(4):
                    ps = ps_pool.tile([128, 1024], F32, name="ps", tag="odeps")
                    base = IL + 1024 * q
                    for sub in range(2):
                        c0 = base + 512 * sub
                        out_sl = ps[:, 512 * sub:512 * sub + 512]
                        for k in range(3):
                            nc.tensor.matmul(
                                out_sl, w_sl_fn(k),
                                rhs[:, c0 - 1 + k:c0 - 1 + k + 512],
                                start=(k == 0), stop=(k == 2))
                    nc.scalar.activation(dst_tile[:, base:base + 1024], ps[:],
                                         func, bias=bias_ap)

            def halo1(t):
                nc.vector.tensor_copy(t[:, IL - 1:IL], t[:, IL:IL + 1])
                nc.vector.tensor_copy(t[:, IR:IR + 1], t[:, IR - 1:IR])

            # initial fp16 casts (later steps cast in the combine tail)
            for g in range(G):
                nc.vector.tensor_copy(z16[g][:, IL - 1:IR + 1],
                                      z[g][:, IL - 1:IR + 1])
            for step in range(steps):
                us = [[None] * 4 for _ in range(G)]
                w12 = [None] * G
                w123 = [None] * G
                for si in range(4):
                    for g in range(G):
                        if si == 0:
                            rhs = z16[g]
                        else:
                            t_s = tpp.tile([128, W], F16, name="t_s", tag="tp")
                            cs = ks["stage_c"][si]
                            nc.vector.tensor_scalar_mul(
                                t_s[:, IL:IR], us[g][si - 1][:, IL:IR],
                                float(cs))
                            nc.vector.tensor_add(
                                t_s[:, IL:IR], t_s[:, IL:IR],
                                z16[g][:, IL:IR])
                            halo1(t_s)
                            rhs = t_s
                        h_t = hp.tile([128, W], F16, name="h_t", tag="h")
                        conv_block(lambda k, s=si: c1w_sl(s, k), rhs, h_t,
                                   AF.Gelu, c1b[:, 0:1])
                        halo1(h_t)
                        u_t = up.tile([128, W], F16, name="u_t", tag="u")
                        conv_block(c2w_sl, h_t, u_t, AF.Tanh, c2b[:, 0:1])
                        us[g][si] = u_t
                        if si == 0:
                            # scale z by A early (frees the z dependency chain)
                            nc.vector.tensor_scalar_mul(
                                z[g][:, IL:IR], z[g][:, IL:IR], float(A))
                        if si == 1:
                            vt = up.tile([128, W], F16, name="vt", tag="u")
                            nc.vector.tensor_scalar_mul(
                                vt[:, IL:IR], us[g][0][:, IL:IR],
                                float(B1 / B2))
                            nc.vector.tensor_add(
                                vt[:, IL:IR], vt[:, IL:IR],
                                us[g][1][:, IL:IR])
                            w12[g] = vt
                        if si == 2:
                            # w123 = (B2/B4)*w12 + (B3/B4)*u3  (off critical path)
                            tmp = up.tile([128, W], F16, name="tmp", tag="u")
                            nc.vector.tensor_scalar_mul(
                                tmp[:, IL:IR], us[g][2][:, IL:IR],
                                float(B3 / B4))
                            nc.vector.tensor_scalar_mul(
                                w12[g][:, IL:IR], w12[g][:, IL:IR],
                                float(B2 / B4))
                            wt = up.tile([128, W], F16, name="wt", tag="u")
                            nc.vector.tensor_add(
                                wt[:, IL:IR], w12[g][:, IL:IR],
                                tmp[:, IL:IR])
                            w123[g] = wt
                        if si == 3:
                            # tail: v = w123 + u4 ; z' = B4*v + A*z ; recast
                            v = up.tile([128, W], F16, name="v", tag="u")
                            nc.vector.tensor_add(
                                v[:, IL:IR], w123[g][:, IL:IR],
                                us[g][3][:, IL:IR])
                            nc.vector.scalar_tensor_tensor(
                                z[g][:, IL:IR], v[:, IL:IR], float(B4),
                                z[g][:, IL:IR], OP.mult, OP.add)
                            halo1(z[g])
                            if step < steps - 1:
                                nc.vector.tensor_copy(
                                    z16[g][:, IL - 1:IR + 1],
                                    z[g][:, IL - 1:IR + 1])

        # final cast of z for the wavelet level-0 input, with full edge halos
        zw16 = [state.tile([128, W], F16, name=f"zw16_{g}", tag=f"zw16_{g}")
                for g in range(G)]
        for g in range(G):
            nc.vector.tensor_copy(zw16[g][:, IL:IR], z[g][:, IL:IR])
            bcast_cols(zw16[g][:, 0:4], zw16[g][:, IL:IL + 1], 4)
            bcast_cols(zw16[g][:, IR:IR + 4], zw16[g][:, IR - 1:IR], 4)

        # ================= Wavelet levels =================
        det_stores = {}
        with ExitStack() as wv_ctx:
            apool = wv_ctx.enter_context(tc.tile_pool(name="approx", bufs=1))
            vpool = wv_ctx.enter_context(tc.tile_pool(name="vfil", bufs=2))
            dpool = wv_ctx.enter_context(tc.tile_pool(name="diag", bufs=2))
            spool = wv_ctx.enter_context(tc.tile_pool(name="mlp", bufs=2))
            ps_cv = wv_ctx.enter_context(
                tc.tile_pool(name="pscv", bufs=2, space="PSUM"))
            ps_sm = wv_ctx.enter_context(
                tc.tile_pool(name="pssm", bufs=1, space="PSUM"))

            apA = [apool.tile([128, W], F16, name=f"apA{g}", tag=f"apA{g}")
                   for g in range(G)]
            apB = [apool.tile([128, W], F16, name=f"apB{g}", tag=f"apB{g}")
                   for g in range(G)]

            for lev in range(LEVEL):
                src_l = [zw16, apB, apA][lev]
                dst = [apB, apA, None][lev]
                for g in range(G):
                    # ---- per-group stat MLP (fp32 matmuls) ----
                    m_t = spool.tile([128, 1], F32, name="m_t", tag=f"m{g}")
                    nc.vector.tensor_reduce(
                        m_t[:], src_l[g][:, IL:IR],
                        axis=mybir.AxisListType.X, op=OP.add)
                    mlp_ps = ps_sm.tile([128, 8], F32, name="mlp_ps",
                                        tag="mlp_ps", bufs=2)
                    nc.tensor.matmul(mlp_ps[:, 0:1], statw[:], m_t[:],
                                     start=True, stop=True)
                    stat_t = spool.tile([128, 1], F32, name="stat_t",
                                        tag=f"stat{g}")
                    nc.scalar.activation(stat_t[:], mlp_ps[:, 0:1], AF.Gelu,
                                         bias=statb[:, 0:1],
                                         scale=float(1.0 / (L * WS[lev])))
                    for s in range(2):
                        nc.tensor.matmul(
                            mlp_ps[:, 1 + s:2 + s],
                            gen1w[64 * s:64 * s + 64, :],
                            stat_t[64 * s:64 * s + 64, 0:1],
                            start=True, stop=True)
                    g1_t = spool.tile([128, 2], F32, name="g1_t", tag=f"g1{g}")
                    nc.scalar.activation(g1_t[:], mlp_ps[:, 1:3], AF.Gelu,
                                         bias=gen1b[:, 0:1])
                    nc.tensor.matmul(mlp_ps[0:16, 3:5], gen2w[:], g1_t[:],
                                     start=True, stop=True)
                    gall = spool.tile([16, 2], F32, name="gall", tag=f"gall{g}")
                    nc.scalar.activation(gall[:], mlp_ps[0:16, 3:5],
                                         AF.Identity, bias=gen2b[:, 0:1])
                    g_store = nc.sync.dma_start(d_g[lev][:, 2 * g:2 * g + 2],
                                                gall[:])
                    # broadcast per-sample filter rows back from DRAM
                    vf = vpool.tile([128, 2 * FL], F32, name="vf", tag="vf")
                    for s in range(2):
                        r = 2 * g + s
                        src_b = (d_g[lev][:, r:r + 1].transpose([1, 0])
                                 .partition_broadcast(64).squeeze(1))
                        ld_v = nc.sync.dma_start(vf[64 * s:64 * s + 64, :],
                                                 src_b)
                        bass_rust.add_dep_helper(ld_v.ins, g_store.ins,
                                                 reason="filter roundtrip")
                    dblk = dpool.tile([128, 2 * FL * 128], F16,
                                      name="dblk", tag="dblk")
                    lo_sc = float(WS[lev + 1] / WS[lev]) if lev < 2 else 1.0
                    hi_sc = float(WD[lev] / WS[lev])
                    for k in range(2 * FL):
                        if dst is None and k < FL:
                            continue   # level 2 needs no lo filters
                        nc.vector.tensor_scalar(
                            dblk[:, 128 * k:128 * k + 128], idm[:],
                            vf[:, k:k + 1], lo_sc if k < FL else hi_sc,
                            OP.mult, OP.mult)
                    det_sb = dpool.tile([128, L], F32, name="det_sb",
                                        tag="det_sb")
                    for q in range(8):          # 512-wide psum tiles
                        base = 512 * q
                        c0 = base          # rhs tile col for tap k: c0 + k
                        if dst is not None:
                            pl = ps_cv.tile([128, 512], F32, name="pl",
                                            tag="pslo")
                            for k in range(FL):
                                nc.tensor.matmul(
                                    pl[:], dblk[:, 128 * k:128 * k + 128],
                                    src_l[g][:, c0 + k:c0 + k + 512],
                                    start=(k == 0), stop=(k == FL - 1))
                            nc.scalar.copy(
                                dst[g][:, IL + base:IL + base + 512], pl[:])
                        ph = ps_cv.tile([128, 512], F32, name="ph", tag="pshi")
                        for k in range(FL):
                            nc.tensor.matmul(
                                ph[:], dblk[:, 128 * (FL + k):
                                            128 * (FL + k) + 128],
                                src_l[g][:, c0 + k:c0 + k + 512],
                                start=(k == 0), stop=(k == FL - 1))
                        nc.scalar.mul(det_sb[:, base:base + 512], ph[:],
                                      float(1.0 / WD[lev]))
                    if lev < 2:
                        st = nc.gpsimd.dma_start(d_det[lev, g], det_sb[:])
                    else:
                        st = nc.gpsimd.dma_start(d_enh[2][g], det_sb[:])
                    det_stores[(lev, g)] = [st]
                    if dst is not None:
                        bcast_cols(dst[g][:, 0:4], dst[g][:, IL:IL + 1], 4)
                        bcast_cols(dst[g][:, IR:IR + 4], dst[g][:, IR - 1:IR], 4)

        # ================= Fusion =================
        with ExitStack() as fu_ctx:
            dets = fu_ctx.enter_context(tc.tile_pool(name="dets", bufs=2))
            fbuf = fu_ctx.enter_context(tc.tile_pool(name="fbuf", bufs=2))
            fps = fu_ctx.enter_context(
                tc.tile_pool(name="fps", bufs=4, space="PSUM"))

            # zero 1-col halos for the zero-padded gate conv
            for g in range(G):
                nc.vector.memset(z[g][:, IL - 1:IL], 0.0)
                nc.vector.memset(z[g][:, IR:IR + 1], 0.0)

            for lev in (2, 1, 0):
                for g in range(G):
                    det_t = dets.tile([128, L], F32, name="det_t", tag="det")
                    src_ap = d_enh[2][g] if lev == 2 else d_det[lev, g]
                    ld = nc.gpsimd.dma_start(det_t[:], src_ap)
                    for st in det_stores[(lev, g)]:
                        bass_rust.add_dep_helper(ld.ins, st.ins,
                                                 reason="det roundtrip")
                    cur16 = fbuf.tile([128, W], F16, name="cur16",
                                      tag="cur16")
                    nc.vector.tensor_copy(cur16[:, IL - 1:IR + 1],
                                          z[g][:, IL - 1:IR + 1])
                    if lev < 2:
                        a1 = fbuf.tile([32, L], F16, name="a1", tag="a1")
                        for q in range(4):
                            pa = fps.tile([128, 1024], F32, name="pa", tag="fps")
                            for sub in range(2):
                                c0 = IL + 1024 * q + 512 * sub
                                nc.tensor.matmul(
                                    pa[0:32, 512 * sub:512 * sub + 512],
                                    attw1[:, 32 * lev:32 * lev + 32],
                                    cur16[:, c0:c0 + 512],
                                    start=True, stop=True)
                            nc.scalar.activation(
                                a1[:, 1024 * q:1024 * q + 1024], pa[0:32, :],
                                AF.Gelu, bias=attb1[:, lev:lev + 1])
                        att = fbuf.tile([128, L], F16, name="att", tag="att")
                        for q in range(4):
                            pa = fps.tile([128, 1024], F32, name="pa2",
                                          tag="fps")
                            for sub in range(2):
                                sl = slice(512 * sub, 512 * sub + 512)
                                nc.tensor.matmul(
                                    pa[:, sl],
                                    attw2[:, 128 * lev:128 * lev + 128],
                                    a1[:, 1024 * q + 512 * sub:
                                       1024 * q + 512 * sub + 512],
                                    start=True, stop=True)
                            nc.scalar.activation(
                                att[:, 1024 * q:1024 * q + 1024], pa[:],
                                AF.Sigmoid, bias=attb2[:, lev:lev + 1])
                        de = fbuf.tile([128, L], F32, name="de", tag="de")
                        nc.vector.scalar_tensor_tensor(
                            de[:], att[:], 1.0, det_t[:], OP.add, OP.mult)
                        nc.gpsimd.dma_start(d_enh[lev][g], de[:])
                        det_eff = de
                    else:
                        det_eff = det_t
                    gate = fbuf.tile([128, L], F16, name="gate", tag="gate",
                                     bufs=1)
                    for q in range(4):
                        pg = fps.tile([128, 1024], F32, name="pg", tag="fps")
                        for sub in range(2):
                            sl = slice(512 * sub, 512 * sub + 512)
                            c0 = IL + 1024 * q + 512 * sub
                            for k in range(3):
                                nc.tensor.matmul(
                                    pg[:, sl], gatew_sl(lev, k),
                                    cur16[:, c0 - 1 + k:c0 - 1 + k + 512],
                                    start=(k == 0), stop=(k == 2))
                        nc.scalar.activation(
                            gate[:, 1024 * q:1024 * q + 1024], pg[:],
                            AF.Sigmoid, bias=gateb[:, lev:lev + 1])
                    gd = fbuf.tile([128, L], F32, name="gd", tag="gd", bufs=1)
                    nc.vector.tensor_mul(gd[:], gate[:], det_eff[:])
                    nc.vector.tensor_add(z[g][:, IL:IR], z[g][:, IL:IR], gd[:])
            for g in range(G):
                nc.gpsimd.dma_start(d_cur[g], z[g][:, IL:IR])

    nc.compile()
    return nc


# ---------------- host packing ----------------

def _pack_inputs(inputs):
    f32 = np.float32
    bf = np.float16
    x = np.ascontiguousarray(np.asarray(inputs["x"], f32))
    w1 = np.asarray(inputs["ode_c1_w"], f32)
    b1 = np.asarray(inputs["ode_c1_b"], f32)
    w2 = np.asarray(inputs["ode_c2_w"], f32)
    b2 = np.asarray(inputs["ode_c2_b"], f32)
    damp = float(np.asarray(inputs["damp"]))
    stat_w = np.asarray(inputs["stat_w"], f32)
    stat_b = np.asarray(inputs["stat_b"], f32)
    gen1_w = np.asarray(inputs["gen1_w"], f32)
    gen1_b = np.asarray(inputs["gen1_b"], f32)
    gen2_w = np.asarray(inputs["gen2_w"], f32)
    gen2_b = np.asarray(inputs["gen2_b"], f32)
    gate_w = np.asarray(inputs["gate_w"], f32)
    gate_b = np.asarray(inputs["gate_b"], f32)
    att_w1 = np.asarray(inputs["att_w1"], f32)
    att_b1 = np.asarray(inputs["att_b1"], f32)
    att_w2 = np.asarray(inputs["att_w2"], f32)
    att_b2 = np.asarray(inputs["att_b2"], f32)

    ks = _rk4_constants(damp)

    def bd(m):  # block-diag 2x of [64,64] -> [128,128]
        out = np.zeros((128, 128), f32)
        out[:64, :64] = m
        out[64:, 64:] = m
        return out

    c1w = np.zeros((128, 4, 3, 128), f32)
    for si, a in enumerate(ks["stage_scale"]):
        for k in range(3):
            c1w[:, si, k, :] = bd(w1[:, :, k].T * a)
    c2w = np.zeros((128, 3, 128), f32)
    for k in range(3):
        c2w[:, k, :] = bd(w2[:, :, k].T)
    gatew = np.zeros((128, 3, 3, 128), f32)
    for lev in range(3):
        for k in range(3):
            gatew[:, lev, k, :] = bd(gate_w[lev, :, :, k].T)
    attw1 = np.zeros((128, 2, 32), f32)
    attw2 = np.zeros((32, 2, 128), f32)
    for lev in range(2):
        for s in range(2):
            attw1[64 * s:64 * s + 64, lev, 16 * s:16 * s + 16] = att_w1[lev].T
            attw2[16 * s:16 * s + 16, lev, 64 * s:64 * s + 64] = att_w2[lev].T
    statw = bd(stat_w.T)
    gen1w = np.zeros((128, 128), f32)
    gen1w[:64] = gen1_w.T
    gen1w[64:] = gen1_w.T

    dup = lambda v: np.tile(v, 2).astype(f32)[:, None]
    base = {
        "c1w": np.ascontiguousarray(c1w.reshape(128, -1)).astype(bf),
        "c2w": np.ascontiguousarray(c2w.reshape(128, -1)).astype(bf),
        "c1b": dup(b1),
        "c2b": dup(b2),
        "statw": statw,
        "statb": dup(stat_b),
        "gen1w": gen1w,
        "gen1b": np.ascontiguousarray(gen1_b.astype(f32)[:, None]),
        "gen2w": np.ascontiguousarray(gen2_w.T.astype(f32)),
        "gen2b": np.ascontiguousarray(gen2_b.astype(f32)[:, None]),
        "gatew": np.ascontiguousarray(gatew.reshape(128, -1)).astype(bf),
        "gateb": np.ascontiguousarray(
            np.stack([np.tile(gate_b[l], 2) for l in range(3)], 1).astype(f32)),
        "attw1": np.ascontiguousarray(attw1.reshape(128, -1)).astype(bf),
        "attb1": np.ascontiguousarray(
            np.stack([np.tile(att_b1[l], 2) for l in range(2)], 1).astype(f32)),
        "attw2": np.ascontiguousarray(attw2.reshape(32, -1)).astype(bf),
        "attb2": np.ascontiguousarray(
            np.stack([np.tile(att_b2[l], 2) for l in range(2)], 1).astype(f32)),
        "idmat": np.eye(128, dtype=f32),
    }
    in_maps = []
    for core in range(NCORES):
        xg = np.zeros((G, 128, L), f32)
        for g in range(G):
            for s in range(2):
                xg[g, 64 * s:64 * s + 64] = x[BS * core + 2 * g + s]
        m = dict(base)
        m["x"] = xg
        in_maps.append(m)
    return in_maps, damp


def _host_ortho(los, his):
    f32 = np.float32
    ortho = f32(0.0)
    for lev in range(LEVEL):
        lo = los[lev].astype(f32)
        lo_pad = np.pad(lo, ((0, 0), (1, 1)))
        lo_smooth = np.abs(lo_pad[:, 1:] - lo_pad[:, :-1]).mean(dtype=f32)
        nrm = np.sqrt((lo.astype(np.float64) ** 2).sum(1, keepdims=True))
        lo_n = (lo / (nrm.astype(f32) + f32(1e-8))).astype(f32)
        shift = f32(0.0)
        for s in range(1, 4):
            shift += np.abs(lo_n[:, :, None] *
                            np.roll(lo_n, s, axis=1)[:, None, :]).mean(dtype=f32)
        amp = np.abs((lo_n ** 2).sum(1) - f32(1.0)).mean(dtype=f32)
        ortho = ortho + f32(REG) * (shift + amp) + f32(0.1) * lo_smooth
    return np.float32(ortho)


_PROG_CACHE = {}


def kernel(**inputs) -> tuple:
    in_maps, damp = _pack_inputs(inputs)
    if damp not in _PROG_CACHE:
        _PROG_CACHE[damp] = build_program(damp)
    nc = _PROG_CACHE[damp]
    res = run_bass_kernel_spmd(nc, in_maps, list(range(NCORES)))
    results = res.results

    f32 = np.float32
    cur = np.zeros((B, C, L), f32)
    enh = [np.zeros((B, C, L), f32) for _ in range(3)]
    los = np.zeros((LEVEL, B, FL), f32)
    his = np.zeros((LEVEL, B, FL), f32)
    for core in range(NCORES):
        r = results[core]
        for g in range(G):
            for s in range(2):
                bidx = BS * core + 2 * g + s
                sl = slice(64 * s, 64 * s + 64)
                cur[bidx] = r["out_cur"][g, sl]
                for i in range(3):
                    enh[i][bidx] = r[f"out_enh{i}"][g, sl]
                los[:, bidx, :] = r["out_g"][:, 0:FL, 2 * g + s]
                his[:, bidx, :] = r["out_g"][:, FL:2 * FL, 2 * g + s]
    ortho = _host_ortho(los, his)
    return (cur, enh[0], enh[1], enh[2], ortho, los, his)


# revision 19
# speedup vs baseline: 1.0441x; 1.0441x over previous
"""Trainium2 Bass kernel for nn_AdvancedWaveletDecomp (self-contained).

Pure data parallel over batch B=32 across 8 NeuronCores (4 samples/core).
On each core, 2 samples are packed per 128-partition group (rows 0:64 =
sample A channels, 64:128 = sample B), giving 2 groups per core.

 - ODE (10 RK4 steps): K=3 channel-mixing convs = 3 accumulating 128x128
   block-diagonal matmuls over shifted views of halo-padded SBUF tiles.
   RK4 is refolded so stage inputs are t_i = a_i*(z + c_i*u_j) with
   host-precomputed constants (a_i folded into per-stage conv1 weights).
   State z stays fp32; stage rhs / gelu / tanh tensors are bf16.
 - Dynamic wavelet filters: per-level stat MLP in fp32 matmuls (columns =
   samples); the [16,4] filter block is PE-transposed, broadcast into
   per-sample [128,8] tables, expanded to diagonal lhsT matrices, and the
   8-tap filters run as accumulating f32r diagonal matmuls over shifted
   edge-padded windows.
 - Cross-scale fusion: 1x1 att convs + K=3 zero-padded gate convs as f32r
   matmuls, sigmoid/gelu on ACT, gating on DVE.
 - ortho and the tiny reshapes/scalar reductions run on host numpy.
"""
from contextlib import ExitStack

import numpy as np
import ml_dtypes

import concourse.bass as bass
import concourse.bacc as bacc
import concourse.tile as tile
import concourse.mybir as mybir
import bass_rust
from concourse.bass_utils import run_bass_kernel_spmd

B, C, L = 32, 64, 4096
LEVEL, FL, DIM, HID, MID = 3, 8, 64, 64, 16
REG = 0.01
ODE_H, ODE_STEPS = 0.1, 10
NCORES = 8
BS = B // NCORES     # samples per core
G = BS // 2          # 2-sample groups per core

F32 = mybir.dt.float32
F32R = mybir.dt.float32r
BF16 = mybir.dt.bfloat16
F16 = mybir.dt.float16

IL, IR = 4, 4 + L    # interior columns of padded tiles
W = L + 8            # padded tile width

AF = mybir.ActivationFunctionType
OP = mybir.AluOpType

TileCtx = tile.TileContext

# power-of-2 scaling keeps fp16 wavelet operands out of subnormals
WS = [1.0, 64.0, 2097152.0]        # approx storage scale per level
WD = [64.0, 524288.0, 70368744177664.0]  # det psum scale per level (2^6,2^19,2^46)



def _rk4_constants(damp: float):
    """Exact z' coefficients over [z, u1..u4] plus truncated stage scales."""
    h = ODE_H
    z = np.array([1.0, 0, 0, 0, 0], np.float64)
    e = np.eye(5)
    t1 = z
    k1 = e[1] - damp * t1
    t2 = z + (h / 2) * k1
    k2 = e[2] - damp * t2
    t3 = z + (h / 2) * k2
    k3 = e[3] - damp * t3
    t4 = z + h * k3
    k4 = e[4] - damp * t4
    zc = z + (h / 6) * (k1 + 2 * k2 + 2 * k3 + k4)
    A, B1, B2, B3, B4 = [float(v) for v in zc]
    a2 = 1.0 - damp * h / 2
    a3 = 1.0 - damp * h / 2 * a2
    a4 = 1.0 - damp * h * a3
    return dict(A=A, Bs=[B1, B2, B3, B4], stage_scale=[1.0, a2, a3, a4],
                stage_c=[None, (h / 2) / a2, (h / 2) / a3, h / a4])


def build_program(damp: float, steps: int = ODE_STEPS):
    ks = _rk4_constants(damp)
    A = ks["A"]
    B1, B2, B3, B4 = ks["Bs"]
    nc = bacc.Bacc("TRN2", target_bir_lowering=False, debug=False)

    # ---------------- DRAM I/O ----------------
    def din(name, shape, dt):
        return nc.dram_tensor(name, list(shape), dt, kind="ExternalInput").ap()

    d_x = din("x", [G, 128, L], F32)
    d_c1w = din("c1w", [128, 4 * 3 * 128], F16)       # [cin_s, (stage,k,cout_s)]
    d_c2w = din("c2w", [128, 3 * 128], F16)           # [din_s, (k,cout_s)]
    d_c1b = din("c1b", [128, 1], F32)
    d_c2b = din("c2b", [128, 1], F32)
    d_statw = din("statw", [128, 128], F32)
    d_statb = din("statb", [128, 1], F32)
    d_gen1w = din("gen1w", [128, 128], F32)            # both halves = gen1_w.T
    d_gen1b = din("gen1b", [128, 1], F32)
    d_gen2w = din("gen2w", [128, 16], F32)
    d_gen2b = din("gen2b", [16, 1], F32)
    d_gatew = din("gatew", [128, 3 * 3 * 128], F16)    # [cin_s, (lev,k,cout_s)]
    d_gateb = din("gateb", [128, 3], F32)
    d_attw1 = din("attw1", [128, 2 * 32], F16)         # [cin_s, (lev, m_s)]
    d_attb1 = din("attb1", [32, 2], F32)
    d_attw2 = din("attw2", [32, 2 * 128], F16)        # [m_s, (lev, cout_s)]
    d_attb2 = din("attb2", [128, 2], F32)
    d_id = din("idmat", [128, 128], F32)
    d_sel = din("selw", [2, 128], F32)

    d_cur = nc.dram_tensor("out_cur", [G, 128, L], F32, kind="ExternalOutput").ap()
    d_enh = [nc.dram_tensor(f"out_enh{i}", [G, 128, L], F32,
                            kind="ExternalOutput").ap() for i in range(3)]
    d_g = nc.dram_tensor("out_g", [LEVEL, 16, 2 * G], F32,
                         kind="ExternalOutput").ap()
    d_det = nc.dram_tensor("det_scr", [2, G, 128, L], F32).ap()  # lev 0,1 scratch

    def bcast_cols(dst, src_col, n):
        # replicate one column into n columns (edge padding)
        nc.vector.tensor_copy(dst, src_col.broadcast_to((src_col.shape[0], n)))

    with TileCtx(nc) as tc, ExitStack() as top:
        const = top.enter_context(tc.tile_pool(name="const", bufs=1))
        state = top.enter_context(tc.tile_pool(name="state", bufs=1))

        # ---- load constants ----
        def cload(dname, dram_ap, shape, dt):
            t = const.tile(shape, dt, name=f"c_{dname}", tag=f"c_{dname}")
            nc.sync.dma_start(t[:], dram_ap)
            return t

        c1w = cload("c1w", d_c1w, [128, 4 * 3 * 128], F16)
        c2w = cload("c2w", d_c2w, [128, 3 * 128], F16)
        c1b = cload("c1b", d_c1b, [128, 1], F32)
        c2b = cload("c2b", d_c2b, [128, 1], F32)
        statw = cload("statw", d_statw, [128, 128], F32)
        statb = cload("statb", d_statb, [128, 1], F32)
        gen1w = cload("gen1w", d_gen1w, [128, 128], F32)
        gen1b = cload("gen1b", d_gen1b, [128, 1], F32)
        gen2w = cload("gen2w", d_gen2w, [128, 16], F32)
        gen2b = cload("gen2b", d_gen2b, [16, 1], F32)
        gatew = cload("gatew", d_gatew, [128, 3 * 3 * 128], F16)
        gateb = cload("gateb", d_gateb, [128, 3], F32)
        attw1 = cload("attw1", d_attw1, [128, 2 * 32], F16)
        attb1 = cload("attb1", d_attb1, [32, 2], F32)
        attw2 = cload("attw2", d_attw2, [32, 2 * 128], F16)
        attb2 = cload("attb2", d_attb2, [128, 2], F32)
        idm = cload("idmat", d_id, [128, 128], F32)
        selw = cload("selw", d_sel, [2, 128], F32)

        def c1w_sl(stage, k):
            o = (stage * 3 + k) * 128
            return c1w[:, o:o + 128]

        def c2w_sl(k):
            return c2w[:, k * 128:(k + 1) * 128]

        def gatew_sl(lev, k):
            o = (lev * 3 + k) * 128
            return gatew[:, o:o + 128]

        # ---- persistent state tiles ----
        z = [state.tile([128, W], F32, name=f"z{g}", tag=f"z{g}")
             for g in range(G)]
        for g in range(G):
            nc.sync.dma_start(z[g][:, IL:IR], d_x[g])
            nc.vector.tensor_copy(z[g][:, IL - 1:IL], z[g][:, IL:IL + 1])
            nc.vector.tensor_copy(z[g][:, IR:IR + 1], z[g][:, IR - 1:IR])

        # ================= ODE =================
        with ExitStack() as ode_ctx:
            ps_pool = ode_ctx.enter_context(
                tc.tile_pool(name="odeps", bufs=4, space="PSUM"))
            zp16 = ode_ctx.enter_context(tc.tile_pool(name="z16p", bufs=1))
            hp = ode_ctx.enter_context(tc.tile_pool(name="hp", bufs=2))
            tpp = ode_ctx.enter_context(tc.tile_pool(name="tpp", bufs=2))
            up = ode_ctx.enter_context(tc.tile_pool(name="up", bufs=10))

            z16 = [zp16.tile([128, W], F16, name=f"z16_{g}", tag=f"z16_{g}")
                   for g in range(G)]

            def conv_block(w_sl_fn, rhs, dst_tile, func, bias_ap):
                # full-width K=3 conv + activation; rhs/dst are [128, W] tiles
                for q in range(4):
                    ps = ps_pool.tile([128, 1024], F32, name="ps", tag="odeps")
                    base = IL + 1024 * q
                    for sub in range(2):
                        c0 = base + 512 * sub
                        out_sl = ps[:, 512 * sub:512 * sub + 512]
                        for k in range(3):
                            nc.tensor.matmul(
                                out_sl, w_sl_fn(k),
                                rhs[:, c0 - 1 + k:c0 - 1 + k + 512],
                                start=(k == 0), stop=(k == 2))
                    nc.scalar.activation(dst_tile[:, base:base + 1024], ps[:],
                                         func, bias=bias_ap)

            def halo1(t):
                nc.vector.tensor_copy(t[:, IL - 1:IL], t[:, IL:IL + 1])
                nc.vector.tensor_copy(t[:, IR:IR + 1], t[:, IR - 1:IR])

            # initial fp16 casts (later steps cast in the combine tail)
            for g in range(G):
                nc.vector.tensor_copy(z16[g][:, IL - 1:IR + 1],
                                      z[g][:, IL - 1:IR + 1])
            for step in range(steps):
                us = [[None] * 4 for _ in range(G)]
                w12 = [None] * G
                w123 = [None] * G
                for si in range(4):
                    for g in range(G):
                        if si == 0:
                            rhs = z16[g]
                        else:
                            t_s = tpp.tile([128, W], F16, name="t_s", tag="tp")
                            cs = ks["stage_c"][si]
                            nc.vector.tensor_scalar_mul(
                                t_s[:, IL:IR], us[g][si - 1][:, IL:IR],
                                float(cs))
                            nc.vector.tensor_add(
                                t_s[:, IL:IR], t_s[:, IL:IR],
                                z16[g][:, IL:IR])
                            halo1(t_s)
                            rhs = t_s
                        h_t = hp.tile([128, W], F16, name="h_t", tag="h")
                        conv_block(lambda k, s=si: c1w_sl(s, k), rhs, h_t,
                                   AF.Gelu, c1b[:, 0:1])
                        halo1(h_t)
                        u_t = up.tile([128, W], F16, name="u_t", tag="u")
                        conv_block(c2w_sl, h_t, u_t, AF.Tanh, c2b[:, 0:1])
                        us[g][si] = u_t
                        if si == 0:
                            # scale z by A early (frees the z dependency chain)
                            nc.vector.tensor_scalar_mul(
                                z[g][:, IL:IR], z[g][:, IL:IR], float(A))
                        if si == 1:
                            vt = up.tile([128, W], F16, name="vt", tag="u")
                            nc.vector.tensor_scalar_mul(
                                vt[:, IL:IR], us[g][0][:, IL:IR],
                                float(B1 / B2))
                            nc.vector.tensor_add(
                                vt[:, IL:IR], vt[:, IL:IR],
                                us[g][1][:, IL:IR])
                            w12[g] = vt
                        if si == 2:
                            # w123 = (B2/B4)*w12 + (B3/B4)*u3  (off critical path)
                            tmp = up.tile([128, W], F16, name="tmp", tag="u")
                            nc.vector.tensor_scalar_mul(
                                tmp[:, IL:IR], us[g][2][:, IL:IR],
                                float(B3 / B4))
                            nc.vector.tensor_scalar_mul(
                                w12[g][:, IL:IR], w12[g][:, IL:IR],
                                float(B2 / B4))
                            wt = up.tile([128, W], F16, name="wt", tag="u")
                            nc.vector.tensor_add(
                                wt[:, IL:IR], w12[g][:, IL:IR],
                                tmp[:, IL:IR])
                            w123[g] = wt
                        if si == 3:
                            # tail: v = w123 + u4 ; z' = B4*v + A*z ; recast
                            v = up.tile([128, W], F16, name="v", tag="u")
                            nc.vector.tensor_add(
                                v[:, IL:IR], w123[g][:, IL:IR],
                                us[g][3][:, IL:IR])
                            nc.vector.scalar_tensor_tensor(
                                z[g][:, IL:IR], v[:, IL:IR], float(B4),
                                z[g][:, IL:IR], OP.mult, OP.add)
                            halo1(z[g])
                            if step < steps - 1:
                                nc.vector.tensor_copy(
                                    z16[g][:, IL - 1:IR + 1],
                                    z[g][:, IL - 1:IR + 1])

        # final cast of z for the wavelet level-0 input, with full edge halos
        zw16 = [state.tile([128, W], F16, name=f"zw16_{g}", tag=f"zw16_{g}")
                for g in range(G)]
        for g in range(G):
            nc.vector.tensor_copy(zw16[g][:, IL:IR], z[g][:, IL:IR])
            bcast_cols(zw16[g][:, 0:4], zw16[g][:, IL:IL + 1], 4)
            bcast_cols(zw16[g][:, IR:IR + 4], zw16[g][:, IR - 1:IR], 4)

        # ================= Wavelet levels =================
        det_stores = {}
        with ExitStack() as wv_ctx:
            apool = wv_ctx.enter_context(tc.tile_pool(name="approx", bufs=1))
            vpool = wv_ctx.enter_context(tc.tile_pool(name="vfil", bufs=2))
            dpool = wv_ctx.enter_context(tc.tile_pool(name="diag", bufs=2))
            spool = wv_ctx.enter_context(tc.tile_pool(name="mlp", bufs=2))
            ps_cv = wv_ctx.enter_context(
                tc.tile_pool(name="pscv", bufs=2, space="PSUM"))
            ps_sm = wv_ctx.enter_context(
                tc.tile_pool(name="pssm", bufs=1, space="PSUM"))

            apA = [apool.tile([128, W], F16, name=f"apA{g}", tag=f"apA{g}")
                   for g in range(G)]
            apB = [apool.tile([128, W], F16, name=f"apB{g}", tag=f"apB{g}")
                   for g in range(G)]

            for lev in range(LEVEL):
                src_l = [zw16, apB, apA][lev]
                dst = [apB, apA, None][lev]
                for g in range(G):
                    # ---- per-group stat MLP (fp32 matmuls) ----
                    m_t = spool.tile([128, 1], F32, name="m_t", tag=f"m{g}")
                    nc.vector.tensor_reduce(
                        m_t[:], src_l[g][:, IL:IR],
                        axis=mybir.AxisListType.X, op=OP.add)
                    mlp_ps = ps_sm.tile([128, 40], F32, name="mlp_ps",
                                        tag="mlp_ps", bufs=2)
                    nc.tensor.matmul(mlp_ps[:, 0:1], statw[:], m_t[:],
                                     start=True, stop=True)
                    stat_t = spool.tile([128, 1], F32, name="stat_t",
                                        tag=f"stat{g}")
                    nc.scalar.activation(stat_t[:], mlp_ps[:, 0:1], AF.Gelu,
                                         bias=statb[:, 0:1],
                                         scale=float(1.0 / (L * WS[lev])))
                    for s in range(2):
                        nc.tensor.matmul(
                            mlp_ps[:, 1 + s:2 + s],
                            gen1w[64 * s:64 * s + 64, :],
                            stat_t[64 * s:64 * s + 64, 0:1],
                            start=True, stop=True)
                    g1_t = spool.tile([128, 2], F32, name="g1_t", tag=f"g1{g}")
                    nc.scalar.activation(g1_t[:], mlp_ps[:, 1:3], AF.Gelu,
                                         bias=gen1b[:, 0:1])
                    nc.tensor.matmul(mlp_ps[0:16, 3:5], gen2w[:], g1_t[:],
                                     start=True, stop=True)
                    gall = spool.tile([16, 2], F32, name="gall", tag=f"gall{g}")
                    nc.scalar.activation(gall[:], mlp_ps[0:16, 3:5],
                                         AF.Identity, bias=gen2b[:, 0:1])
                    nc.sync.dma_start(d_g[lev][:, 2 * g:2 * g + 2], gall[:])
                    # on-chip broadcast: transpose [16,2]->[2,16], then
                    # V[c_s, k] = gT[sample(c_s), k] via a selector matmul
                    nc.tensor.transpose(mlp_ps[0:2, 5:21], gall[:],
                                        idm[0:16, 0:16])
                    gT = spool.tile([2, 16], F32, name="gT", tag=f"gT{g}")
                    nc.scalar.copy(gT[:], mlp_ps[0:2, 5:21])
                    nc.tensor.matmul(mlp_ps[:, 21:37], selw[:], gT[:],
                                     start=True, stop=True)
                    vf = vpool.tile([128, 2 * FL], F32, name="vf", tag="vf")
                    nc.scalar.copy(vf[:], mlp_ps[:, 21:37])
                    dblk = dpool.tile([128, 2 * FL * 128], F16,
                                      name="dblk", tag="dblk")
                    lo_sc = float(WS[lev + 1] / WS[lev]) if lev < 2 else 1.0
                    hi_sc = float(WD[lev] / WS[lev])
                    for k in range(2 * FL):
                        if dst is None and k < FL:
                            continue   # level 2 needs no lo filters
                        nc.vector.tensor_scalar(
                            dblk[:, 128 * k:128 * k + 128], idm[:],
                            vf[:, k:k + 1], lo_sc if k < FL else hi_sc,
                            OP.mult, OP.mult)
                    det_sb = dpool.tile([128, L], F32, name="det_sb",
                                        tag="det_sb")
                    for q in range(8):          # 512-wide psum tiles
                        base = 512 * q
                        c0 = base          # rhs tile col for tap k: c0 + k
                        if dst is not None:
                            pl = ps_cv.tile([128, 512], F32, name="pl",
                                            tag="pslo")
                            for k in range(FL):
                                nc.tensor.matmul(
                                    pl[:], dblk[:, 128 * k:128 * k + 128],
                                    src_l[g][:, c0 + k:c0 + k + 512],
                                    start=(k == 0), stop=(k == FL - 1))
                            nc.scalar.copy(
                                dst[g][:, IL + base:IL + base + 512], pl[:])
                        ph = ps_cv.tile([128, 512], F32, name="ph", tag="pshi")
                        for k in range(FL):
                            nc.tensor.matmul(
                                ph[:], dblk[:, 128 * (FL + k):
                                            128 * (FL + k) + 128],
                                src_l[g][:, c0 + k:c0 + k + 512],
                                start=(k == 0), stop=(k == FL - 1))
                        nc.scalar.mul(det_sb[:, base:base + 512], ph[:],
                                      float(1.0 / WD[lev]))
                    if lev < 2:
                        st = nc.gpsimd.dma_start(d_det[lev, g], det_sb[:])
                    else:
                        st = nc.gpsimd.dma_start(d_enh[2][g], det_sb[:])
                    det_stores[(lev, g)] = [st]
                    if dst is not None:
                        bcast_cols(dst[g][:, 0:4], dst[g][:, IL:IL + 1], 4)
                        bcast_cols(dst[g][:, IR:IR + 4], dst[g][:, IR - 1:IR], 4)

        # ================= Fusion =================
        with ExitStack() as fu_ctx:
            dets = fu_ctx.enter_context(tc.tile_pool(name="dets", bufs=2))
            fbuf = fu_ctx.enter_context(tc.tile_pool(name="fbuf", bufs=2))
            fps = fu_ctx.enter_context(
                tc.tile_pool(name="fps", bufs=4, space="PSUM"))

            # zero 1-col halos for the zero-padded gate conv
            for g in range(G):
                nc.vector.memset(z[g][:, IL - 1:IL], 0.0)
                nc.vector.memset(z[g][:, IR:IR + 1], 0.0)

            for lev in (2, 1, 0):
                for g in range(G):
                    det_t = dets.tile([128, L], F32, name="det_t", tag="det")
                    src_ap = d_enh[2][g] if lev == 2 else d_det[lev, g]
                    ld = nc.gpsimd.dma_start(det_t[:], src_ap)
                    for st in det_stores[(lev, g)]:
                        bass_rust.add_dep_helper(ld.ins, st.ins,
                                                 reason="det roundtrip")
                    cur16 = fbuf.tile([128, W], F16, name="cur16",
                                      tag="cur16")
                    nc.vector.tensor_copy(cur16[:, IL - 1:IR + 1],
                                          z[g][:, IL - 1:IR + 1])
                    if lev < 2:
                        a1 = fbuf.tile([32, L], F16, name="a1", tag="a1")
                        for q in range(4):
                            pa = fps.tile([128, 1024], F32, name="pa", tag="fps")
                            for sub in range(2):
                                c0 = IL + 1024 * q + 512 * sub
                                nc.tensor.matmul(
                                    pa[0:32, 512 * sub:512 * sub + 512],
                                    attw1[:, 32 * lev:32 * lev + 32],
                                    cur16[:, c0:c0 + 512],
                                    start=True, stop=True)
                            nc.scalar.activation(
                                a1[:, 1024 * q:1024 * q + 1024], pa[0:32, :],
                                AF.Gelu, bias=attb1[:, lev:lev + 1])
                        att = fbuf.tile([128, L], F16, name="att", tag="att")
                        for q in range(4):
                            pa = fps.tile([128, 1024], F32, name="pa2",
                                          tag="fps")
                            for sub in range(2):
                                sl = slice(512 * sub, 512 * sub + 512)
                                nc.tensor.matmul(
                                    pa[:, sl],
                                    attw2[:, 128 * lev:128 * lev + 128],
                                    a1[:, 1024 * q + 512 * sub:
                                       1024 * q + 512 * sub + 512],
                                    start=True, stop=True)
                            nc.scalar.activation(
                                att[:, 1024 * q:1024 * q + 1024], pa[:],
                                AF.Sigmoid, bias=attb2[:, lev:lev + 1])
                        de = fbuf.tile([128, L], F32, name="de", tag="de")
                        nc.vector.scalar_tensor_tensor(
                            de[:], att[:], 1.0, det_t[:], OP.add, OP.mult)
                        nc.gpsimd.dma_start(d_enh[lev][g], de[:])
                        det_eff = de
                    else:
                        det_eff = det_t
                    gate = fbuf.tile([128, L], F16, name="gate", tag="gate",
                                     bufs=1)
                    for q in range(4):
                        pg = fps.tile([128, 1024], F32, name="pg", tag="fps")
                        for sub in range(2):
                            sl = slice(512 * sub, 512 * sub + 512)
                            c0 = IL + 1024 * q + 512 * sub
                            for k in range(3):
                                nc.tensor.matmul(
                                    pg[:, sl], gatew_sl(lev, k),
                                    cur16[:, c0 - 1 + k:c0 - 1 + k + 512],
                                    start=(k == 0), stop=(k == 2))
                        nc.scalar.activation(
                            gate[:, 1024 * q:1024 * q + 1024], pg[:],
                            AF.Sigmoid, bias=gateb[:, lev:lev + 1])
                    gd = fbuf.tile([128, L], F32, name="gd", tag="gd", bufs=1)
                    nc.vector.tensor_mul(gd[:], gate[:], det_eff[:])
                    nc.vector.tensor_add(z[g][:, IL:IR], z[g][:, IL:IR], gd[:])
            for g in range(G):
                nc.gpsimd.dma_start(d_cur[g], z[g][:, IL:IR])

    nc.compile()
    return nc


# ---------------- host packing ----------------

def _sel():
    s = np.zeros((2, 128), np.float32)
    s[0, 0:64] = 1.0
    s[1, 64:128] = 1.0
    return s


def _pack_inputs(inputs):
    f32 = np.float32
    bf = np.float16
    x = np.ascontiguousarray(np.asarray(inputs["x"], f32))
    w1 = np.asarray(inputs["ode_c1_w"], f32)
    b1 = np.asarray(inputs["ode_c1_b"], f32)
    w2 = np.asarray(inputs["ode_c2_w"], f32)
    b2 = np.asarray(inputs["ode_c2_b"], f32)
    damp = float(np.asarray(inputs["damp"]))
    stat_w = np.asarray(inputs["stat_w"], f32)
    stat_b = np.asarray(inputs["stat_b"], f32)
    gen1_w = np.asarray(inputs["gen1_w"], f32)
    gen1_b = np.asarray(inputs["gen1_b"], f32)
    gen2_w = np.asarray(inputs["gen2_w"], f32)
    gen2_b = np.asarray(inputs["gen2_b"], f32)
    gate_w = np.asarray(inputs["gate_w"], f32)
    gate_b = np.asarray(inputs["gate_b"], f32)
    att_w1 = np.asarray(inputs["att_w1"], f32)
    att_b1 = np.asarray(inputs["att_b1"], f32)
    att_w2 = np.asarray(inputs["att_w2"], f32)
    att_b2 = np.asarray(inputs["att_b2"], f32)

    ks = _rk4_constants(damp)

    def bd(m):  # block-diag 2x of [64,64] -> [128,128]
        out = np.zeros((128, 128), f32)
        out[:64, :64] = m
        out[64:, 64:] = m
        return out

    c1w = np.zeros((128, 4, 3, 128), f32)
    for si, a in enumerate(ks["stage_scale"]):
        for k in range(3):
            c1w[:, si, k, :] = bd(w1[:, :, k].T * a)
    c2w = np.zeros((128, 3, 128), f32)
    for k in range(3):
        c2w[:, k, :] = bd(w2[:, :, k].T)
    gatew = np.zeros((128, 3, 3, 128), f32)
    for lev in range(3):
        for k in range(3):
            gatew[:, lev, k, :] = bd(gate_w[lev, :, :, k].T)
    attw1 = np.zeros((128, 2, 32), f32)
    attw2 = np.zeros((32, 2, 128), f32)
    for lev in range(2):
        for s in range(2):
            attw1[64 * s:64 * s + 64, lev, 16 * s:16 * s + 16] = att_w1[lev].T
            attw2[16 * s:16 * s + 16, lev, 64 * s:64 * s + 64] = att_w2[lev].T
    statw = bd(stat_w.T)
    gen1w = np.zeros((128, 128), f32)
    gen1w[:64] = gen1_w.T
    gen1w[64:] = gen1_w.T

    dup = lambda v: np.tile(v, 2).astype(f32)[:, None]
    base = {
        "c1w": np.ascontiguousarray(c1w.reshape(128, -1)).astype(bf),
        "c2w": np.ascontiguousarray(c2w.reshape(128, -1)).astype(bf),
        "c1b": dup(b1),
        "c2b": dup(b2),
        "statw": statw,
        "statb": dup(stat_b),
        "gen1w": gen1w,
        "gen1b": np.ascontiguousarray(gen1_b.astype(f32)[:, None]),
        "gen2w": np.ascontiguousarray(gen2_w.T.astype(f32)),
        "gen2b": np.ascontiguousarray(gen2_b.astype(f32)[:, None]),
        "gatew": np.ascontiguousarray(gatew.reshape(128, -1)).astype(bf),
        "gateb": np.ascontiguousarray(
            np.stack([np.tile(gate_b[l], 2) for l in range(3)], 1).astype(f32)),
        "attw1": np.ascontiguousarray(attw1.reshape(128, -1)).astype(bf),
        "attb1": np.ascontiguousarray(
            np.stack([np.tile(att_b1[l], 2) for l in range(2)], 1).astype(f32)),
        "attw2": np.ascontiguousarray(attw2.reshape(32, -1)).astype(bf),
        "attb2": np.ascontiguousarray(
            np.stack([np.tile(att_b2[l], 2) for l in range(2)], 1).astype(f32)),
        "idmat": np.eye(128, dtype=f32),
        "selw": np.stack([np.tile(np.array([1.0, 0.0], f32)[s:s + 1].repeat(64),
                                  2)[:128] for s in range(2)]
                         ).astype(f32) if False else _sel(),
    }
    in_maps = []
    for core in range(NCORES):
        xg = np.zeros((G, 128, L), f32)
        for g in range(G):
            for s in range(2):
                xg[g, 64 * s:64 * s + 64] = x[BS * core + 2 * g + s]
        m = dict(base)
        m["x"] = xg
        in_maps.append(m)
    return in_maps, damp


def _host_ortho(los, his):
    f32 = np.float32
    ortho = f32(0.0)
    for lev in range(LEVEL):
        lo = los[lev].astype(f32)
        lo_pad = np.pad(lo, ((0, 0), (1, 1)))
        lo_smooth = np.abs(lo_pad[:, 1:] - lo_pad[:, :-1]).mean(dtype=f32)
        nrm = np.sqrt((lo.astype(np.float64) ** 2).sum(1, keepdims=True))
        lo_n = (lo / (nrm.astype(f32) + f32(1e-8))).astype(f32)
        shift = f32(0.0)
        for s in range(1, 4):
            shift += np.abs(lo_n[:, :, None] *
                            np.roll(lo_n, s, axis=1)[:, None, :]).mean(dtype=f32)
        amp = np.abs((lo_n ** 2).sum(1) - f32(1.0)).mean(dtype=f32)
        ortho = ortho + f32(REG) * (shift + amp) + f32(0.1) * lo_smooth
    return np.float32(ortho)


_PROG_CACHE = {}


def kernel(**inputs) -> tuple:
    in_maps, damp = _pack_inputs(inputs)
    if damp not in _PROG_CACHE:
        _PROG_CACHE[damp] = build_program(damp)
    nc = _PROG_CACHE[damp]
    res = run_bass_kernel_spmd(nc, in_maps, list(range(NCORES)))
    results = res.results

    f32 = np.float32
    cur = np.zeros((B, C, L), f32)
    enh = [np.zeros((B, C, L), f32) for _ in range(3)]
    los = np.zeros((LEVEL, B, FL), f32)
    his = np.zeros((LEVEL, B, FL), f32)
    for core in range(NCORES):
        r = results[core]
        for g in range(G):
            for s in range(2):
                bidx = BS * core + 2 * g + s
                sl = slice(64 * s, 64 * s + 64)
                cur[bidx] = r["out_cur"][g, sl]
                for i in range(3):
                    enh[i][bidx] = r[f"out_enh{i}"][g, sl]
                los[:, bidx, :] = r["out_g"][:, 0:FL, 2 * g + s]
                his[:, bidx, :] = r["out_g"][:, FL:2 * FL, 2 * g + s]
    ortho = _host_ortho(los, his)
    return (cur, enh[0], enh[1], enh[2], ortho, los, his)
